# revision 11
# baseline (speedup 1.0000x reference)
"""Batched normalized-gram kernel for 8 TRN2 NeuronCores.

reference:  x (64, 2, 512, 512) fp32
    x0 = x[:, 0]                               (B=64, V=512, F=512)
    n  = sqrt(sum(x0^2, axis=(0, 2)))          (V,)
    out[b] = (x0[b] @ x0[b].T) / outer(n, n)   (B, V, V)

Since gram[b,i,j]/(n_i n_j) == (x0[b,i,:]/n_i) . (x0[b,j,:]/n_j), the host
prescales rows by 1/n once (cheap: one pass over 67MB) and the device work
is a pure batched symmetric matmul out[b] = y[b] @ y[b].T.

Sharding: data-parallel over batch — 8 batches per core.  The host ships
yT[b] = y[b].T (F-major) so both matmul operands stream straight from SBUF
with no on-device transpose.
"""

import numpy as np

B, T, V, F = 64, 2, 512, 512
NCORES = 8
BPC = B // NCORES  # batches per core

_NC = None


def _build_nc():
    import concourse.mybir as mybir
    import concourse.tile as tile
    from concourse import bacc

    f32 = mybir.dt.float32
    f16 = mybir.dt.float16

    nc = bacc.Bacc(target_bir_lowering=False)
    yT = nc.declare_dram_parameter("yT", [BPC, F, V], f16, isOutput=False)
    out = nc.declare_dram_parameter("out", [BPC, V, V], f32, isOutput=True)

    with tile.TileContext(nc) as tc:
        with (
            tc.tile_pool(name="inp", bufs=3) as inp_pool,
            tc.tile_pool(name="psum", bufs=8, space="PSUM") as psum_pool,
            tc.tile_pool(name="outp", bufs=6) as out_pool,
        ):
            for b in range(BPC):
                # yT[b] is (F, V); SBUF holds it as 4 partition-chunks of
                # [128, V] laid side by side: yt[:, ki*V:(ki+1)*V].
                yt = inp_pool.tile([128, 4 * V], f16)
                for ki in range(4):
                    nc.sync.dma_start(
                        out=yt[:, ki * V : (ki + 1) * V],
                        in_=yT[b, ki * 128 : (ki + 1) * 128, :],
                    )
                for mi in range(4):
                    ps = psum_pool.tile([128, V], f32)
                    for ki in range(4):
                        chunk = yt[:, ki * V : (ki + 1) * V]
                        nc.tensor.matmul(
                            ps,
                            lhsT=chunk[:, mi * 128 : (mi + 1) * 128],
                            rhs=chunk,
                            start=(ki == 0),
                            stop=(ki == 3),
                        )
                    ot = out_pool.tile([128, V], f32)
                    nc.vector.tensor_copy(out=ot, in_=ps)
                    nc.sync.dma_start(
                        out=out[b, mi * 128 : (mi + 1) * 128, :], in_=ot
                    )
    if not nc.is_finalized():
        nc.finalize()
    return nc


def _get_nc():
    global _NC
    if _NC is None:
        _NC = _build_nc()
    return _NC


def _prep_shards(x: np.ndarray) -> np.ndarray:
    x = np.ascontiguousarray(np.asarray(x, dtype=np.float32))
    x0 = x[:, 0]  # (B, V, F)
    ss = np.einsum("bvf,bvf->v", x0, x0, optimize=True)
    inv_n = (1.0 / np.sqrt(ss)).astype(np.float32)
    y = x0 * inv_n[None, :, None]
    # (B, F, V) contiguous so each core's operand streams with unit stride.
    # fp16 is safe here: |y| <= ~0.05 (well inside fp16 normal range) and it
    # halves the input DMA while running the PE at full rate.
    return np.ascontiguousarray(np.transpose(y, (0, 2, 1)).astype(np.float16))


def kernel(x: np.ndarray, _trace: bool = False, _trace_out: list | None = None):
    from concourse.bass_utils import run_bass_kernel_spmd

    yT = _prep_shards(x)
    nc = _get_nc()
    in_maps = [{"yT": yT[c * BPC : (c + 1) * BPC]} for c in range(NCORES)]
    res = run_bass_kernel_spmd(
        nc, in_maps, core_ids=list(range(NCORES)), trace=_trace
    )
    if _trace_out is not None:
        _trace_out.append(res)
    return np.concatenate(
        [np.asarray(res.results[c]["out"]) for c in range(NCORES)], axis=0
    )


# revision 12
# speedup vs baseline: 1.0202x; 1.0202x over previous
"""Batched normalized-gram kernel for 8 TRN2 NeuronCores.

reference:  x (64, 2, 512, 512) fp32
    x0 = x[:, 0]                               (B=64, V=512, F=512)
    n  = sqrt(sum(x0^2, axis=(0, 2)))          (V,)
    out[b] = (x0[b] @ x0[b].T) / outer(n, n)   (B, V, V)

Since gram[b,i,j]/(n_i n_j) == (x0[b,i,:]/n_i) . (x0[b,j,:]/n_j), the host
prescales rows by 1/n once and the device work is a pure batched symmetric
matmul out[b] = y[b] @ y[b].T.

Device-side tricks:
  * operands shipped as fp16 (|y| <= ~0.05, comfortably normal) — halves
    input DMA, full-rate PE, fp32 PSUM accumulation keeps rel err ~2e-4.
  * out[b] is symmetric, and the reference is *exactly* symmetric (same
    products, same summation order), so the device computes only the upper
    block-triangle (row-block mi covers columns mi*128..511) and the host
    mirrors the lower blocks.  -37.5% output DMA, -37.5% PE work.

Sharding: data-parallel over batch — 8 batches per core, no collectives.
"""

import numpy as np

B, T, V, F = 64, 2, 512, 512
NCORES = 8
BPC = B // NCORES  # batches per core
NBLK = V // 128  # 4 row-blocks

_NC = None


def _build_nc():
    import concourse.mybir as mybir
    import concourse.tile as tile
    from concourse import bacc

    f32 = mybir.dt.float32
    f16 = mybir.dt.float16

    nc = bacc.Bacc(target_bir_lowering=False)
    yT = nc.declare_dram_parameter("yT", [BPC, F, V], f16, isOutput=False)
    out = nc.declare_dram_parameter("out", [BPC, V, V], f32, isOutput=True)

    # upper-triangle segment offsets inside the per-batch output tile:
    # row-block mi holds columns mi*128..511 (N = 512 - 128*mi)
    seg_off = [0]
    for mi in range(NBLK):
        seg_off.append(seg_off[-1] + V - 128 * mi)
    seg_total = seg_off[-1]  # 1280

    with tile.TileContext(nc) as tc:
        with (
            tc.tile_pool(name="inp", bufs=4) as inp_pool,
            tc.tile_pool(name="psum", bufs=8, space="PSUM") as psum_pool,
            tc.tile_pool(name="outp", bufs=3) as out_pool,
        ):
            for b in range(BPC):
                # yT[b] is (F, V); SBUF holds 4 partition-chunks of [128, V]
                # side by side.  Per-chunk DMAs let the first matmuls start
                # before the whole batch has landed.
                yt = inp_pool.tile([128, NBLK * V], f16)
                for ki in range(NBLK):
                    nc.sync.dma_start(
                        out=yt[:, ki * V : (ki + 1) * V],
                        in_=yT[b, ki * 128 : (ki + 1) * 128, :],
                    )
                ot = out_pool.tile([128, seg_total], f32)
                for mi in range(NBLK):
                    n_cols = V - 128 * mi
                    ps = psum_pool.tile([128, n_cols], f32, tag="ps")
                    for ki in range(NBLK):
                        chunk = yt[:, ki * V : (ki + 1) * V]
                        nc.tensor.matmul(
                            ps,
                            lhsT=chunk[:, mi * 128 : (mi + 1) * 128],
                            rhs=chunk[:, mi * 128 :],
                            start=(ki == 0),
                            stop=(ki == NBLK - 1),
                        )
                    seg = ot[:, seg_off[mi] : seg_off[mi] + n_cols]
                    nc.vector.tensor_copy(out=seg, in_=ps)
                    nc.sync.dma_start(
                        out=out[b, mi * 128 : (mi + 1) * 128, mi * 128 :],
                        in_=seg,
                    )
    if not nc.is_finalized():
        nc.finalize()
    return nc


def _get_nc():
    global _NC
    if _NC is None:
        _NC = _build_nc()
    return _NC


def _prep_shards(x: np.ndarray) -> np.ndarray:
    x = np.ascontiguousarray(np.asarray(x, dtype=np.float32))
    x0 = x[:, 0]  # (B, V, F)
    ss = np.einsum("bvf,bvf->v", x0, x0, optimize=True)
    inv_n = (1.0 / np.sqrt(ss)).astype(np.float32)
    y = x0 * inv_n[None, :, None]
    # (B, F, V) contiguous so each core's operand streams with unit stride
    return np.ascontiguousarray(np.transpose(y, (0, 2, 1)).astype(np.float16))


def kernel(x: np.ndarray, _trace: bool = False, _trace_out: list | None = None):
    from concourse.bass_utils import run_bass_kernel_spmd

    yT = _prep_shards(x)
    nc = _get_nc()
    in_maps = [{"yT": yT[c * BPC : (c + 1) * BPC]} for c in range(NCORES)]
    res = run_bass_kernel_spmd(
        nc, in_maps, core_ids=list(range(NCORES)), trace=_trace
    )
    if _trace_out is not None:
        _trace_out.append(res)
    full = np.concatenate(
        [np.asarray(res.results[c]["out"]) for c in range(NCORES)], axis=0
    )
    # device wrote only the upper block-triangle; mirror it down
    for mi in range(NBLK):
        for nj in range(mi + 1, NBLK):
            full[:, nj * 128 : (nj + 1) * 128, mi * 128 : (mi + 1) * 128] = (
                np.swapaxes(
                    full[:, mi * 128 : (mi + 1) * 128, nj * 128 : (nj + 1) * 128],
                    1,
                    2,
                )
            )
    return full


# revision 13
# speedup vs baseline: 1.1151x; 1.0931x over previous
"""Batched normalized-gram kernel for 8 TRN2 NeuronCores.

reference:  x (64, 2, 512, 512) fp32
    x0 = x[:, 0]                               (B=64, V=512, F=512)
    n  = sqrt(sum(x0^2, axis=(0, 2)))          (V,)
    out[b] = (x0[b] @ x0[b].T) / outer(n, n)   (B, V, V)

Since gram[b,i,j]/(n_i n_j) == (x0[b,i,:]/n_i) . (x0[b,j,:]/n_j), the host
prescales rows by 1/n once and the device work is a pure batched symmetric
matmul out[b] = y[b] @ y[b].T.

Device-side tricks:
  * operands shipped as fp16 (|y| <= ~0.05, comfortably normal) — halves
    input DMA, full-rate PE, fp32 PSUM accumulation keeps rel err ~2e-4.
  * out[b] is symmetric, and the reference is *exactly* symmetric (same
    products, same summation order), so the device computes only the upper
    block-triangle (row-block mi covers columns mi*128..511) and the host
    mirrors the lower blocks.  -37.5% output DMA, -37.5% PE work.

Sharding: data-parallel over batch — 8 batches per core, no collectives.
"""

import numpy as np

B, T, V, F = 64, 2, 512, 512
NCORES = 8
BPC = B // NCORES  # batches per core
NBLK = V // 128  # 4 row-blocks

_NC = None


def _build_nc():
    import concourse.mybir as mybir
    import concourse.tile as tile
    from concourse import bacc

    f32 = mybir.dt.float32
    f16 = mybir.dt.float16

    nc = bacc.Bacc(target_bir_lowering=False)
    yT = nc.declare_dram_parameter("yT", [BPC, F, V], f16, isOutput=False)
    out = nc.declare_dram_parameter("out", [BPC, V, V], f32, isOutput=True)

    # upper-triangle segment offsets inside the per-batch output tile:
    # row-block mi holds columns mi*128..511 (N = 512 - 128*mi)
    seg_off = [0]
    for mi in range(NBLK):
        seg_off.append(seg_off[-1] + V - 128 * mi)
    seg_total = seg_off[-1]  # 1280

    with tile.TileContext(nc) as tc:
        with (
            tc.tile_pool(name="inp", bufs=10) as inp_pool,
            tc.tile_pool(name="psum", bufs=8, space="PSUM") as psum_pool,
            tc.tile_pool(name="outp", bufs=3) as out_pool,
        ):
            for b in range(BPC):
                # yT[b] is (F, V): four partition-chunks of [128, V], one
                # tile each so matmuls depend only on the chunk they read.
                # Input DMAs ride the SP HWDGE ring; output DMAs ride the
                # ACT ring — two independent FIFOs running concurrently.
                chunks = []
                for ki in range(NBLK):
                    ck = inp_pool.tile([128, V], f16, tag="ck")
                    nc.sync.dma_start(
                        out=ck, in_=yT[b, ki * 128 : (ki + 1) * 128, :]
                    )
                    chunks.append(ck)
                ot = out_pool.tile([128, seg_total], f32)
                for mi in range(NBLK):
                    n_cols = V - 128 * mi
                    ps = psum_pool.tile([128, n_cols], f32, tag="ps")
                    for ki in range(NBLK):
                        nc.tensor.matmul(
                            ps,
                            lhsT=chunks[ki][:, mi * 128 : (mi + 1) * 128],
                            rhs=chunks[ki][:, mi * 128 :],
                            start=(ki == 0),
                            stop=(ki == NBLK - 1),
                        )
                    seg = ot[:, seg_off[mi] : seg_off[mi] + n_cols]
                    nc.vector.tensor_copy(out=seg, in_=ps)
                    nc.scalar.dma_start(
                        out=out[b, mi * 128 : (mi + 1) * 128, mi * 128 :],
                        in_=seg,
                    )
    if not nc.is_finalized():
        nc.finalize()
    return nc


def _get_nc():
    global _NC
    if _NC is None:
        _NC = _build_nc()
    return _NC


def _prep_shards(x: np.ndarray) -> np.ndarray:
    x = np.ascontiguousarray(np.asarray(x, dtype=np.float32))
    x0 = x[:, 0]  # (B, V, F)
    ss = np.einsum("bvf,bvf->v", x0, x0, optimize=True)
    inv_n = (1.0 / np.sqrt(ss)).astype(np.float32)
    y = x0 * inv_n[None, :, None]
    # (B, F, V) contiguous so each core's operand streams with unit stride
    return np.ascontiguousarray(np.transpose(y, (0, 2, 1)).astype(np.float16))


def kernel(x: np.ndarray, _trace: bool = False, _trace_out: list | None = None):
    from concourse.bass_utils import run_bass_kernel_spmd

    yT = _prep_shards(x)
    nc = _get_nc()
    in_maps = [{"yT": yT[c * BPC : (c + 1) * BPC]} for c in range(NCORES)]
    res = run_bass_kernel_spmd(
        nc, in_maps, core_ids=list(range(NCORES)), trace=_trace
    )
    if _trace_out is not None:
        _trace_out.append(res)
    full = np.concatenate(
        [np.asarray(res.results[c]["out"]) for c in range(NCORES)], axis=0
    )
    # device wrote only the upper block-triangle; mirror it down
    for mi in range(NBLK):
        for nj in range(mi + 1, NBLK):
            full[:, nj * 128 : (nj + 1) * 128, mi * 128 : (mi + 1) * 128] = (
                np.swapaxes(
                    full[:, mi * 128 : (mi + 1) * 128, nj * 128 : (nj + 1) * 128],
                    1,
                    2,
                )
            )
    return full
